# revision 1
# baseline (speedup 1.0000x reference)
"""Trainium2 Bass kernel for DGLGraphConv-style message passing.

  m = feat[src] * edge_w[:, None]          # [E, D] messages
  h = segment_sum(m, dst, N)               # [N, D]
  out = h @ W + b                          # [N, D]

Strategy (8 NeuronCores, SPMD, no collectives):
  * Host sorts edges by dst and splits the node space into 8 contiguous
    ranges (12500 nodes each) -> each core owns all edges of its nodes.
  * Within a core, nodes are sorted by in-degree (desc) and packed into
    128-node tiles in ELL layout: tile t gets S_t slots per node
    (S_t = max degree in tile, shared schedule across cores so the SPMD
    program is identical; ~2% slot padding).
  * Device per tile: indirect-DMA gather feat_bf16[src] -> [128, S_t, 64],
    DVE multiply by edge_w (bf16, written d-major), DVE reduce over slots
    (f32), PE transpose + linear (f32), DVE bias add, and a batched
    indirect-DMA row scatter into the core-local output.
  * All data-dependent structure (degree sort, slot tables, scatter rows)
    lives in host-precomputed index arrays, so one program serves all cores.
"""

import numpy as np
import ml_dtypes

N_NODES = 100000
N_EDGES = 1600000
D = 64
C = 8
P = 128
NPC = N_NODES // C            # 12500 nodes per core
T = (NPC + P - 1) // P        # 98 tiles per core
NPAD = T * P                  # 12544
CHUNK_COLS = 192              # max slot-columns per gather super-chunk

_cache = {}


def _build_tables(src, dst, edge_w):
    """Host preprocessing: ELL tables per core + shared S_t schedule."""
    order = np.argsort(dst, kind="stable")
    src_s = src[order]
    w_s = edge_w[order]
    counts = np.bincount(dst, minlength=N_NODES)
    rowptr = np.zeros(N_NODES + 1, dtype=np.int64)
    np.cumsum(counts, out=rowptr[1:])

    per_core = []
    S_t_all = np.zeros((C, T), dtype=np.int64)
    for c in range(C):
        lo = c * NPC
        deg = counts[lo:lo + NPC]
        degp = np.concatenate([deg, np.full(NPAD - NPC, -1, dtype=deg.dtype)])
        nperm = np.argsort(-degp, kind="stable")
        sdeg = degp[nperm]
        S_t_all[c] = np.maximum(sdeg.reshape(T, P).max(axis=1), 1)
        per_core.append((lo, nperm, np.maximum(sdeg, 0)))

    S_t = S_t_all.max(axis=0)
    S_t = S_t + (S_t % 2)          # even S for 4B-aligned bf16 rows
    off_t = np.zeros(T + 1, dtype=np.int64)
    np.cumsum(S_t, out=off_t[1:])
    SUMS = int(off_t[-1])

    cores = []
    for c in range(C):
        lo, nperm, sdeg = per_core[c]
        gnode = np.where(nperm < NPC, nperm + lo, 0)
        start = np.where(nperm < NPC, rowptr[np.minimum(gnode, N_NODES - 1)], 0)

        idx_all = np.zeros((P, SUMS), dtype=np.int32)
        ew_all = np.zeros((P, SUMS), dtype=ml_dtypes.bfloat16)
        for t in range(T):
            S = int(S_t[t])
            nodes = slice(t * P, (t + 1) * P)
            d = sdeg[nodes][:, None]
            k = np.arange(S)[None, :]
            valid = k < d
            e = start[nodes][:, None] + np.minimum(k, np.maximum(d - 1, 0))
            idx_all[:, off_t[t]:off_t[t + 1]] = np.where(valid, src_s[e], 0)
            ew_all[:, off_t[t]:off_t[t + 1]] = np.where(
                valid, w_s[e], 0.0).astype(ml_dtypes.bfloat16)

        cores.append((idx_all, ew_all, nperm))
    return S_t, off_t, SUMS, cores


def _chunks(S_t, off_t):
    """Greedy tile grouping: consecutive tiles with sum(S_t) <= CHUNK_COLS."""
    groups = []
    t = 0
    while t < T:
        t0 = t
        cols = 0
        while t < T and cols + int(S_t[t]) <= CHUNK_COLS:
            cols += int(S_t[t])
            t += 1
        groups.append((t0, t, cols))
    return groups


def _build_program(S_t, off_t, SUMS):
    import concourse.bass as bass
    import concourse.mybir as mybir
    import concourse.tile as tile
    from concourse import bacc
    from concourse.masks import make_identity

    f32 = mybir.dt.float32
    bf16 = mybir.dt.bfloat16
    i32 = mybir.dt.int32

    nc = bacc.Bacc("TRN2", target_bir_lowering=False, debug=False,
                   num_devices=C)

    featb = nc.dram_tensor("featb", [N_NODES, D], bf16, kind="ExternalInput").ap()
    idx_d = nc.dram_tensor("idx", [P, SUMS], i32, kind="ExternalInput").ap()
    ew_d = nc.dram_tensor("ew", [P, SUMS], bf16, kind="ExternalInput").ap()
    wt_d = nc.dram_tensor("wt", [D, D], f32, kind="ExternalInput").ap()
    brep_d = nc.dram_tensor("brep", [P, D], f32, kind="ExternalInput").ap()
    out_d = nc.dram_tensor("out", [NPAD, D], f32, kind="ExternalOutput").ap()

    groups = _chunks(S_t, off_t)

    with tile.TileContext(nc) as tc:
        with (
            tc.tile_pool(name="const", bufs=1) as cpool,
            tc.tile_pool(name="sb", bufs=3) as pool,
            tc.tile_pool(name="ps", bufs=2, space="PSUM") as pspool,
        ):
            W_sb = cpool.tile([D, D], f32)
            nc.sync.dma_start(W_sb[:], wt_d[:])
            brep_sb = cpool.tile([P, D], f32)
            nc.sync.dma_start(brep_sb[:], brep_d[:])
            ident = cpool.tile([P, P], f32)
            make_identity(nc, ident[:])

            for (t0, t1, cols) in groups:
                o0 = int(off_t[t0])
                ntile = t1 - t0

                idx_sb = pool.tile([P, cols], i32, tag="idx")
                nc.sync.dma_start(idx_sb[:], idx_d[:, o0:o0 + cols])
                ew_sb = pool.tile([P, cols], bf16, tag="ew")
                nc.sync.dma_start(ew_sb[:], ew_d[:, o0:o0 + cols])

                msgs = pool.tile([P, cols * D], bf16, tag="msgs")
                # HW consumes ONE offset per dest partition-row per indirect
                # DMA, so gather one slot-column (128 rows) per instruction.
                for s in range(cols):
                    nc.gpsimd.indirect_dma_start(
                        out=msgs[:, s * D:(s + 1) * D],
                        out_offset=None,
                        in_=featb[:],
                        in_offset=bass.IndirectOffsetOnAxis(
                            ap=idx_sb[:, s:s + 1], axis=0),
                    )

                outc = pool.tile([P, ntile * D], f32, tag="outc")
                for j in range(ntile):
                    t = t0 + j
                    S = int(S_t[t])
                    lo = int(off_t[t]) - o0

                    m3 = msgs[:, lo * D:(lo + S) * D].rearrange(
                        "p (s d) -> p s d", d=D)
                    ewb = ew_sb[:, lo:lo + S].unsqueeze(2).to_broadcast(
                        [P, S, D])
                    wm = pool.tile([P, S * D], bf16, tag="wm")
                    # write d-major: physical layout [D, S] per partition
                    wm_w = wm[:].rearrange("p (d s) -> p s d", s=S)
                    nc.vector.tensor_tensor(
                        out=wm_w, in0=m3, in1=ewb, op=mybir.AluOpType.mult)

                    h = pool.tile([P, D], f32, tag="h")
                    nc.vector.reduce_sum(
                        h[:], wm[:].rearrange("p (d s) -> p d s", s=S),
                        axis=mybir.AxisListType.X)

                    pt = pspool.tile([D, P], f32, tag="pt")
                    nc.tensor.transpose(pt[:], h[:], ident[:])
                    ht = pool.tile([D, P], f32, tag="ht")
                    nc.scalar.copy(ht[:], pt[:])

                    po = pspool.tile([P, D], f32, tag="po")
                    nc.tensor.matmul(po[:], lhsT=ht[:], rhs=W_sb[:],
                                     start=True, stop=True)
                    nc.vector.tensor_tensor(
                        out=outc[:, j * D:(j + 1) * D], in0=po[:],
                        in1=brep_sb[:], op=mybir.AluOpType.add)

                # rows written in degree-sorted tile order; host unpermutes
                for j in range(ntile):
                    t = t0 + j
                    nc.sync.dma_start(
                        out_d[t * P:(t + 1) * P, :],
                        outc[:, j * D:(j + 1) * D])

    nc.compile()
    return nc


def _prepare(feat, edge_w, src, dst, weight, bias):
    feat = np.asarray(feat, dtype=np.float32)
    edge_w = np.asarray(edge_w, dtype=np.float32)
    src = np.asarray(src, dtype=np.int32)
    dst = np.asarray(dst, dtype=np.int32)
    weight = np.asarray(weight, dtype=np.float32)
    bias = np.asarray(bias, dtype=np.float32)

    S_t, off_t, SUMS, cores = _build_tables(src, dst, edge_w)
    feat_bf = feat.astype(ml_dtypes.bfloat16)
    brep = np.ascontiguousarray(np.broadcast_to(bias, (P, D))).astype(np.float32)

    in_maps = []
    perms = []
    for c in range(C):
        idx_all, ew_all, nperm = cores[c]
        in_maps.append({
            "featb": feat_bf,
            "idx": idx_all,
            "ew": ew_all,
            "wt": weight,
            "brep": brep,
        })
        perms.append(nperm)
    return S_t, off_t, SUMS, in_maps, perms


def kernel(feat, edge_w, src, dst, weight, bias, _trace=False):
    S_t, off_t, SUMS, in_maps, perms = _prepare(
        feat, edge_w, src, dst, weight, bias)

    key = (SUMS, S_t.tobytes())
    if key not in _cache:
        _cache[key] = _build_program(S_t, off_t, SUMS)
    nc = _cache[key]

    from concourse.bass_utils import run_bass_kernel_spmd
    res = run_bass_kernel_spmd(nc, in_maps, core_ids=list(range(C)),
                               trace=_trace)
    out = np.empty((N_NODES, D), dtype=np.float32)
    for c in range(C):
        o = np.asarray(res.results[c]["out"])   # rows in degree-sorted order
        nperm = perms[c]
        real = nperm < NPC                      # drop pad-node rows
        out[c * NPC + nperm[real]] = o[np.nonzero(real)[0]]
    if _trace:
        kernel.last_results = res
    return out



# revision 6
# speedup vs baseline: 1.3470x; 1.3470x over previous
"""Trainium2 Bass kernel for DGLGraphConv-style message passing.

  m = feat[src] * edge_w[:, None]          # [E, D] messages
  h = segment_sum(m, dst, N)               # [N, D]
  out = h @ W + b                          # [N, D]

Strategy (8 NeuronCores, SPMD, no collectives):
  * Host sorts edges by dst; node space split into 8 contiguous ranges
    (12500 nodes each) -> each core owns all edges of its nodes.
  * The src node table is split into 4 groups of 25000 rows so gathers can
    use dma_gather (int16 indices, one instruction per ~16K rows instead of
    one indirect DMA per 128 rows -> amortizes the ~1us SWDGE fixed cost).
  * Per (core, group): nodes are sorted by group-in-degree (desc) and packed
    into 128-node tiles in ELL layout (S_t = max degree in tile, schedule
    shared across cores so the SPMD program is identical; ~3% padding).
  * Device per group: big dma_gather of f32 feat rows (256B descriptors),
    DVE multiply by edge_w -> bf16 d-major, DVE reduce over slots (f32),
    PE transpose + linear, bias add (group 0 only).
  * Combine across groups (each has its own node permutation): group 0's
    result is DMA-written to the core output (rows in g0-sorted order, host
    unpermutes); groups 1-3 are dma_scatter_add-ed (CCE f32 add) onto it.
    TileContext tracks DRAM hazards, so the write and the three scatter-adds
    are properly serialized.
"""

import numpy as np

N_NODES = 100000
N_EDGES = 1600000
D = 64
C = 8                          # cores
G = 4                          # src-table groups (int16 gather index limit)
GSZ = N_NODES // G             # 25000 rows per gather group
P = 128
NPC = N_NODES // C             # 12500 nodes per core
T = (NPC + P - 1) // P         # 98 tiles per core
NPAD = T * P                   # 12544
CHUNK_COLS = 64                # slot-columns per dma_gather instruction
                               # (SWDGE packet limit: 8192 idx = 64*128)

_cache = {}


def _build_tables(src, dst, edge_w):
    """Host preprocessing: per-(core,group) ELL tables + shared schedule.

    Returns:
      S_t: [G, T] slot counts (shared across cores)
      off: [G, T+1] column offsets per group
      cores: per core dict with idx16 (wrapped gather indices), ew, scatter
             index arrays and the g0 node permutation.
    """
    order = np.argsort(dst, kind="stable")
    src_s = src[order]
    w_s = edge_w[order]
    dst_s = dst[order]

    # per (core, group) edge lists, still dst-sorted
    per_cg = []
    for c in range(C):
        lo = c * NPC
        m = (dst_s >= lo) & (dst_s < lo + NPC)
        sc, wc, dc = src_s[m], w_s[m], dst_s[m] - lo
        row = []
        for g in range(G):
            mg = (sc >= g * GSZ) & (sc < (g + 1) * GSZ)
            row.append((sc[mg] - g * GSZ, wc[mg], dc[mg]))
        per_cg.append(row)

    # per-(core,group) degree sort; S_t shared across cores per group
    S_t = np.zeros((G, T), dtype=np.int64)
    meta = [[None] * G for _ in range(C)]
    for c in range(C):
        for g in range(G):
            sg, wg, dg = per_cg[c][g]
            deg = np.bincount(dg, minlength=NPC)
            degp = np.concatenate(
                [deg, np.full(NPAD - NPC, -1, dtype=deg.dtype)])
            nperm = np.argsort(-degp, kind="stable")
            sdeg = np.maximum(degp[nperm], 0)
            S_t[g] = np.maximum(
                S_t[g], np.maximum(sdeg.reshape(T, P).max(axis=1), 1))
            rowptr = np.zeros(NPC + 1, dtype=np.int64)
            np.cumsum(deg, out=rowptr[1:])
            meta[c][g] = (sg, wg, nperm, sdeg, rowptr)

    off = np.zeros((G, T + 1), dtype=np.int64)
    np.cumsum(S_t, axis=1, out=off[:, 1:])

    cores = []
    for c in range(C):
        idx16s, ews, scats = [], [], []
        pos0 = None
        for g in range(G):
            sg, wg, nperm, sdeg, rowptr = meta[c][g]
            SUMS = int(off[g, -1])
            start = np.where(nperm < NPC, rowptr[np.minimum(nperm, NPC - 1)], 0)

            idx = np.zeros((SUMS, P), dtype=np.int16)   # [col, p]
            ew = np.zeros((P, SUMS), dtype=np.float32)
            for t in range(T):
                S = int(S_t[g, t])
                nodes = slice(t * P, (t + 1) * P)
                d = sdeg[nodes][:, None]
                k = np.arange(S)[None, :]
                valid = k < d
                e = start[nodes][:, None] + np.minimum(k, np.maximum(d - 1, 0))
                o0, o1 = int(off[g, t]), int(off[g, t + 1])
                idx[o0:o1, :] = np.where(valid, sg[e], 0).T
                ew[:, o0:o1] = np.where(valid, wg[e], 0.0)

            # wrap gather indices: linear position i = col*128 + p lives at
            # wrapped[(i%16), i//16]; replicate across the 8 groups of 16
            # partitions for the 8 Q7 cores.
            lin = idx.reshape(-1)                       # [SUMS*128]
            wrapped = lin.reshape(-1, 16).T             # [16, SUMS*8]
            idx16 = np.tile(wrapped, (8, 1))            # [128, SUMS*8]
            idx16s.append(np.ascontiguousarray(idx16))
            ews.append(ew)

            if g == 0:
                pos0 = np.empty(NPAD, dtype=np.int64)
                pos0[nperm] = np.arange(NPAD)           # padded-id -> g0 row
            else:
                # scatter target rows (g0-order) for this group's y tiles:
                # position i = t*128 + p holds node nperm[i]
                tgt = pos0[nperm].astype(np.int16)      # [NPAD]
                wrapped = tgt.reshape(-1, 16).T         # [16, NPAD/16]
                scats.append(np.ascontiguousarray(np.tile(wrapped, (8, 1))))

        cores.append({
            "idx16": idx16s, "ew": ews, "scat": scats,
            "nperm0": meta[c][0][2],
        })
    return S_t, off, cores


def _chunks(S_t_g, off_g):
    """Greedy grouping of consecutive tiles with sum(S_t) <= CHUNK_COLS."""
    groups = []
    t = 0
    while t < T:
        t0 = t
        cols = 0
        while t < T and cols + int(S_t_g[t]) <= CHUNK_COLS:
            cols += int(S_t_g[t])
            t += 1
        groups.append((t0, t, cols))
    return groups


def _build_program(S_t, off):
    import concourse.bass as bass
    import concourse.mybir as mybir
    import concourse.tile as tile
    from concourse import bacc
    from concourse.masks import make_identity

    f32 = mybir.dt.float32
    bf16 = mybir.dt.bfloat16
    i16 = mybir.dt.int16

    nc = bacc.Bacc("TRN2", target_bir_lowering=False, debug=False,
                   num_devices=C)

    featf = nc.dram_tensor("featf", [N_NODES, D], f32,
                           kind="ExternalInput").ap()
    SUMS = [int(off[g, -1]) for g in range(G)]
    idx_d = [nc.dram_tensor(f"idx{g}", [P, SUMS[g] * 8], i16,
                            kind="ExternalInput").ap() for g in range(G)]
    ew_d = [nc.dram_tensor(f"ew{g}", [P, SUMS[g]], f32,
                           kind="ExternalInput").ap() for g in range(G)]
    scat_d = [nc.dram_tensor(f"scat{g}", [P, NPAD // 16], i16,
                             kind="ExternalInput").ap() for g in range(1, G)]
    wt_d = nc.dram_tensor("wt", [D, D], f32, kind="ExternalInput").ap()
    brep_d = nc.dram_tensor("brep", [P, D], f32, kind="ExternalInput").ap()
    out_d = nc.dram_tensor("out", [NPAD, D], f32, kind="ExternalOutput").ap()

    with tile.TileContext(nc) as tc:
        with (
            tc.tile_pool(name="const", bufs=1) as cpool,
            tc.tile_pool(name="meta", bufs=2) as mpool,
            tc.tile_pool(name="sb", bufs=3) as pool,
            tc.tile_pool(name="yb", bufs=2) as ypool,
            tc.tile_pool(name="ps", bufs=2, space="PSUM") as pspool,
        ):
            W_sb = cpool.tile([D, D], f32)
            nc.sync.dma_start(W_sb[:], wt_d[:])
            brep_sb = cpool.tile([P, D], f32)
            nc.sync.dma_start(brep_sb[:], brep_d[:])
            ident = cpool.tile([P, P], f32)
            make_identity(nc, ident[:])
            scat_sb = []
            for g in range(1, G):
                s = cpool.tile([P, NPAD // 16], i16)
                nc.sync.dma_start(s[:], scat_d[g - 1][:])
                scat_sb.append(s)

            for g in range(G):
                gi = pool  # alias to quiet linters
                idx_sb = mpool.tile([P, SUMS[g] * 8], i16, tag="idx")
                nc.sync.dma_start(idx_sb[:], idx_d[g][:])
                ew_sb = mpool.tile([P, SUMS[g]], f32, tag="ew")
                nc.sync.dma_start(ew_sb[:], ew_d[g][:])

                ybuf = ypool.tile([P, T * D], f32, tag="y")
                src_ap = featf[g * GSZ:(g + 1) * GSZ, :]

                for (t0, t1, cols) in _chunks(S_t[g], off[g]):
                    o0 = int(off[g, t0])
                    nidx = cols * P

                    msgs = pool.tile([P, cols * D], f32, tag="msgs")
                    nc.gpsimd.dma_gather(
                        msgs[:].rearrange("p (c d) -> p c d", d=D),
                        src_ap,
                        idx_sb[:, o0 * 8:(o0 + cols) * 8],
                        nidx, nidx, D,
                        single_packet=False,
                    )

                    for j in range(t1 - t0):
                        t = t0 + j
                        S = int(S_t[g, t])
                        lo = int(off[g, t]) - o0

                        m3 = msgs[:, lo * D:(lo + S) * D].rearrange(
                            "p (s d) -> p s d", d=D)
                        ewb = ew_sb[:, int(off[g, t]):int(off[g, t]) + S] \
                            .unsqueeze(2).to_broadcast([P, S, D])
                        wm = pool.tile([P, S * D], bf16, tag="wm")
                        # write d-major: physical [D, S] per partition
                        wm_w = wm[:].rearrange("p (d s) -> p s d", s=S)
                        nc.vector.tensor_tensor(
                            out=wm_w, in0=m3, in1=ewb,
                            op=mybir.AluOpType.mult)

                        h = pool.tile([P, D], f32, tag="h")
                        nc.vector.reduce_sum(
                            h[:], wm[:].rearrange("p (d s) -> p d s", s=S),
                            axis=mybir.AxisListType.X)

                        pt = pspool.tile([D, P], f32, tag="pt")
                        nc.tensor.transpose(pt[:], h[:], ident[:])
                        ht = pool.tile([D, P], f32, tag="ht")
                        nc.scalar.copy(ht[:], pt[:])

                        po = pspool.tile([P, D], f32, tag="po")
                        nc.tensor.matmul(po[:], lhsT=ht[:], rhs=W_sb[:],
                                         start=True, stop=True)
                        if g == 0:
                            nc.vector.tensor_tensor(
                                out=ybuf[:, t * D:(t + 1) * D], in0=po[:],
                                in1=brep_sb[:], op=mybir.AluOpType.add)
                        else:
                            nc.scalar.copy(
                                ybuf[:, t * D:(t + 1) * D], po[:])

                y3 = ybuf[:].rearrange("p (t d) -> p t d", d=D)
                if g == 0:
                    # rows in g0-sorted order; host unpermutes
                    nc.sync.dma_start(
                        out_d[:].rearrange("(t p) d -> p t d", p=P), y3)
                else:
                    # split to respect the 8192-idx SWDGE packet limit
                    half = (T // 2) * P                       # 6272
                    for (r0, r1) in ((0, half), (half, NPAD)):
                        nc.gpsimd.dma_scatter_add(
                            out_d[:],
                            ybuf[:, (r0 // P) * D:(r1 // P) * D].rearrange(
                                "p (t d) -> p t d", d=D),
                            scat_sb[g - 1][:, r0 // 16:r1 // 16],
                            r1 - r0, r1 - r0, D,
                            single_packet=False)

    nc.compile()
    return nc


def _prepare(feat, edge_w, src, dst, weight, bias):
    feat = np.ascontiguousarray(np.asarray(feat, dtype=np.float32))
    edge_w = np.asarray(edge_w, dtype=np.float32)
    src = np.asarray(src, dtype=np.int32)
    dst = np.asarray(dst, dtype=np.int32)
    weight = np.asarray(weight, dtype=np.float32)
    bias = np.asarray(bias, dtype=np.float32)

    S_t, off, cores = _build_tables(src, dst, edge_w)
    brep = np.ascontiguousarray(
        np.broadcast_to(bias, (P, D))).astype(np.float32)

    in_maps = []
    perms = []
    for c in range(C):
        m = {"featf": feat, "wt": weight, "brep": brep}
        for g in range(G):
            m[f"idx{g}"] = cores[c]["idx16"][g]
            m[f"ew{g}"] = cores[c]["ew"][g]
        for g in range(1, G):
            m[f"scat{g}"] = cores[c]["scat"][g - 1]
        in_maps.append(m)
        perms.append(cores[c]["nperm0"])
    return S_t, off, in_maps, perms


def kernel(feat, edge_w, src, dst, weight, bias, _trace=False):
    S_t, off, in_maps, perms = _prepare(
        feat, edge_w, src, dst, weight, bias)

    key = S_t.tobytes()
    if key not in _cache:
        _cache[key] = _build_program(S_t, off)
    nc = _cache[key]

    from concourse.bass_utils import run_bass_kernel_spmd
    res = run_bass_kernel_spmd(nc, in_maps, core_ids=list(range(C)),
                               trace=_trace)
    out = np.empty((N_NODES, D), dtype=np.float32)
    for c in range(C):
        o = np.asarray(res.results[c]["out"])   # rows in g0-sorted order
        nperm = perms[c]
        real = nperm < NPC                      # drop pad-node rows
        out[c * NPC + nperm[real]] = o[np.nonzero(real)[0]]
    if _trace:
        kernel.last_results = res
    return out
